# revision 37
# baseline (speedup 1.0000x reference)
"""Trainium2 Bass kernel for multi-head self-attention (nn_CrossAttention).

Reference computation (B=2, S=4096, C=512, H=8 heads, Dh=64):
    q = hid @ Wq.T; k = hid @ Wk.T; v = hid @ Wv.T     (per-head split)
    out = softmax(q k^T / sqrt(Dh)) v                   (per head)
    final = concat_heads(out) @ Wo.T + bo

Sharding: batch*head parallel. 16 (batch, head) units over 8 cores ->
each core owns one batch b and two adjacent heads. Each core computes a
*partial* output projection (its two heads' contribution to final[b]);
the host sums 4 partials per batch and adds the bias.

Device-side layout choices:
  - hidden is passed pre-transposed (hidT [C, S]) so the model dim (the
    contraction dim of all projections) lies on SBUF partitions. It is
    DMA'd in S-major pieces so K-projection (and with it the first
    q-chunk's score/exp pipeline) starts ~8us in, not after the full 8MB.
  - q, k are kept transposed on-chip: qT/kT [128=2*Dh, S]. Only kT and
    qT's first chunk are projected up front; the remaining qT chunks and
    all of V are projected *inside* the first q-chunk's kv loop, hidden
    under the ACT-bound exp steady state.
  - scores are computed transposed (sT [kv, q]) so the PV matmul needs no
    transposition of the 16M-element probability matrix; softmax needs no
    max-pass (scores are O(1) by construction) and the denominator comes
    free from an appended ones-column in V.
  - the normalized per-head outputs live stacked in one [128, S] tile so
    the output projection is a single K=128 matmul per 128-row chunk.
  - normalize + output-projection of q-chunk i interleave into q-chunk
    i+1's kv loop (ACT-bound), so only the last chunk's tail is exposed.
  - all matmuls run as float32r (FP22 truncation, full PE rate).
"""

from contextlib import ExitStack

import numpy as np

import concourse.bacc as bacc
import concourse.bass as bass
import concourse.tile as tile
from concourse import mybir
from concourse.masks import make_identity
from concourse.bass_utils import run_bass_kernel_spmd

B, S, C = 2, 4096, 512
H, DH = 8, 64
HL = 2                # heads per core
DL = HL * DH          # 128, local projection width
N_CORES = 8
CC = C // 128         # 4 contraction chunks for projections
NQ = S // 512         # 8 q-chunks of 512
NJ = S // 128         # 32 kv-chunks of 128
NP = 8                # hid DMA pieces along S
PS = S // NP          # 512

F32 = mybir.dt.float32
F32R = mybir.dt.float32r
EXP = mybir.ActivationFunctionType.Exp


def _emit(tc, nc, hidT, wqT, wkT, wvT, woT, outp, reps=1):
    with tc.tile_pool(name="persist", bufs=1) as persist:
        qT = persist.tile([DL, S], F32R)
        kT = persist.tile([DL, S], F32R)
        v0 = persist.tile([128, NJ, DH + 1], F32R)   # V plus ones col, head 0
        v1 = persist.tile([128, NJ, DH + 1], F32R)   # head 1
        wo = persist.tile([DL, C], F32R)
        wo0 = persist.tile([DH, C], F32R)   # per-head copies (base partition 0)
        wo1 = persist.tile([DH, C], F32R)   # for the final-chunk tail path
        oT0 = persist.tile([DH + 1, S], F32)        # out^T accum + rowsum row
        oT1 = persist.tile([DH + 1, S], F32)
        oTn = persist.tile([DL, S], F32R)           # normalized out^T, stacked

        # whole-tile memset to 1.0: data cols 0..63 are overwritten by the V
        # projection copies below; col 64 stays 1.0 (the rowsum ones-column)
        nc.gpsimd.memset(v0[:].bitcast(F32), 1.0)
        nc.gpsimd.memset(v1[:].bitcast(F32), 1.0)
        nc.gpsimd.memset(oT0[:], 0.0)
        nc.gpsimd.memset(oT1[:], 0.0)

        stack = ExitStack()
        hload = stack.enter_context(tc.tile_pool(name="hload", bufs=1))
        wload = stack.enter_context(tc.tile_pool(name="wload", bufs=1))

        hid_sb = hload.tile([128, CC, S], F32R)
        hidT_r = hidT.rearrange("(cc p) s -> p cc s", p=128)
        wq_sb = wload.tile([128, CC, DL], F32R)
        wk_sb = wload.tile([128, CC, DL], F32R)
        wv_sb = wload.tile([128, CC, DL], F32R)

        def load_w(w_sb, w_dram):
            # weight loads ride the Activation HWDGE queue so the SP queue is
            # free for the latency-critical first hid pieces
            nc.scalar.dma_start(
                out=w_sb[:], in_=w_dram.rearrange("(cc p) d -> p cc d", p=128)
            )

        def load_piece(p, split=False):
            # one 3D DMA per piece (all cc chunks) — fewer HWDGE slots
            nc.sync.dma_start(
                out=hid_sb[:, :, p * PS:(p + 1) * PS],
                in_=hidT_r[:, :, p * PS:(p + 1) * PS],
            )

        # DMA issue order follows the critical path: wk+wq and the first hid
        # piece gate the first score/exp; wv gates the first PV; wo's gate
        # only the output projections ~40us in.
        load_w(wk_sb, wkT)                      # ACT queue
        nc.sync.dma_start(                       # SP queue, in parallel
            out=wq_sb[:], in_=wqT.rearrange("(cc p) d -> p cc d", p=128))
        load_piece(0, split=True)
        load_w(wv_sb, wvT)
        for p in range(1, NP):
            load_piece(p)
        nc.scalar.dma_start(out=wo[:], in_=woT[:, :])
        nc.scalar.dma_start(out=wo0[:], in_=woT[0:DH, :])
        nc.scalar.dma_start(out=wo1[:], in_=woT[DH:DL, :])

        with tc.tile_pool(name="scps", bufs=2, space="PSUM") as scps, \
             tc.tile_pool(name="pvps", bufs=1, space="PSUM") as pvps, \
             tc.tile_pool(name="aux", bufs=2, space="PSUM") as auxps, \
             tc.tile_pool(name="ptsb", bufs=4) as ptsb, \
             tc.tile_pool(name="norm", bufs=2) as norm, \
             tc.tile_pool(name="ndram", bufs=2, space="DRAM") as ndram, \
             tc.tile_pool(name="vstage", bufs=2) as vstage, \
             tc.tile_pool(name="otsb", bufs=4) as otsb:

            ident_f32 = vstage.tile([128, 128], F32, name="ident_f32",
                                    tag="identf")
            make_identity(nc, ident_f32[:])
            ident = vstage.tile([128, 128], F32R, name="ident", tag="ident")
            nc.vector.tensor_copy(ident[:], ident_f32[:].bitcast(F32R))

            # ---- projection helpers (auxps doubles as proj & oproj psum) ----
            def proj_qk(dst, w_sb, sc):
                # psum[m,n] = sum_c W[m,c] hid[n,c] = qT/kT[dl, s]
                ps = auxps.tile([128, 512], F32, name="pj", tag="pj")
                for cc in range(CC):
                    nc.tensor.matmul(
                        ps[:],
                        lhsT=w_sb[:, cc, :],
                        rhs=hid_sb[:, cc, sc * 512:(sc + 1) * 512],
                        start=(cc == 0),
                        stop=(cc == CC - 1),
                    )
                nc.vector.tensor_copy(dst[:, sc * 512:(sc + 1) * 512],
                                      ps[:].bitcast(F32R))

            def proj_v(sc):
                # project vT [dl, s] 512-wide (full PE rate; a natural-layout
                # projection would pay the fp32r narrow-output 4x penalty),
                # then PE-transpose each 128-block back to natural [s, dl]
                ps = auxps.tile([128, 512], F32, name="pj", tag="pj")
                for cc in range(CC):
                    nc.tensor.matmul(
                        ps[:],
                        lhsT=wv_sb[:, cc, :],
                        rhs=hid_sb[:, cc, sc * 512:(sc + 1) * 512],
                        start=(cc == 0),
                        stop=(cc == CC - 1),
                    )
                vs = vstage.tile([128, 512], F32R, name="vs", tag="vs")
                nc.vector.tensor_copy(vs[:], ps[:].bitcast(F32R))
                pt_ = auxps.tile([128, 512], F32, name="pj", tag="pj")
                for i in range(4):
                    nc.tensor.transpose(pt_[:, i * 128:(i + 1) * 128].bitcast(F32R),
                                        vs[:, i * 128:(i + 1) * 128],
                                        ident[:])
                ptr = pt_[:].bitcast(F32R).rearrange("p (j d) -> p j d", j=4)
                nc.vector.tensor_copy(v0[:, 4 * sc:4 * sc + 4, 0:DH],
                                      ptr[:, :, 0:DH])
                nc.vector.tensor_copy(v1[:, 4 * sc:4 * sc + 4, 0:DH],
                                      ptr[:, :, DH:DL])

            # PE warm-up: ~5us of dummy matmuls with no DMA dependency (they
            # read whatever is in qT) so the clock-gate ramp finishes while
            # the first hid piece is still in flight
            for w in range(10):
                wst = scps.tile([128, 1024], F32, name="st", tag="st")
                nc.tensor.matmul(wst[:, 0:512], lhsT=qT[0:DH, 0:128],
                                 rhs=qT[0:DH, 0:512], start=True, stop=True)

            # up-front: only kT chunks 0-1 and qT chunk 0 (each gated by one
            # 512-wide hid piece); later kT chunks are projected inside the
            # first kv loop just ahead of the scores that read them
            proj_qk(kT, wk_sb, 0)
            proj_qk(qT, wq_sb, 0)
            proj_qk(kT, wk_sb, 1)
            k_emitted = 2

            # deferred into q-chunk 0's kv loop:
            deferred_a = [lambda sc=sc: proj_qk(qT, wq_sb, sc)
                          for sc in range(1, NQ)]

            def emit_norm(qc):
                # per-head: reciprocal of the rowsum slice, partition-reshaped
                # (via DRAM, which is flat) so the slow DVE divide runs on all
                # 128 lanes; then broadcast back and scale outT into the
                # stacked oTn
                qo = qc * 512
                for h, oT in enumerate((oT0, oT1)):
                    srow = ndram.tile([1, 512], F32, name="srow", tag="sr")
                    nc.sync.dma_start(out=srow[:], in_=oT[DH:DH + 1, qo:qo + 512])
                    rs = norm.tile([128, 4], F32, name="rs", tag="rs")
                    nc.sync.dma_start(
                        out=rs[:], in_=srow[0, :].rearrange("(p f) -> p f", p=128))
                    nc.vector.reciprocal(rs[:], rs[:])
                    rrow = ndram.tile([1, 512], F32, name="rrow", tag="rr")
                    nc.sync.dma_start(
                        out=rrow[0, :].rearrange("(p f) -> p f", p=128), in_=rs[:])
                    rb = norm.tile([DH, 512], F32, name="rb", tag="rb")
                    r0 = rrow[0, :]
                    bcast = bass.AP(tensor=r0.tensor, offset=r0.offset,
                                    ap=[[0, DH]] + list(r0.ap))
                    nc.sync.dma_start(out=rb[:], in_=bcast)
                    nc.vector.tensor_mul(oTn[h * DH:(h + 1) * DH, qo:qo + 512],
                                         oT[0:DH, qo:qo + 512].bitcast(F32R),
                                         rb[:].bitcast(F32R))

            def emit_tail(qc):
                # final q-chunk: skip the normalize->oproj pipeline (whose DMA
                # broadcast dance would sit exposed at the kernel end). Project
                # each head's un-normalized out^T, then scale by the reciprocal
                # rowsums as per-partition scalars (s IS the partition dim of
                # the projected output) and sum the heads on DVE.
                qo = qc * 512
                # rowsum rows [1,512] -> per-partition scalars [128,4] via a
                # DRAM roundtrip ("(f p)" order puts s%128 on partitions,
                # matching the projected output rows)
                srow = ndram.tile([2, 512], F32, name="tsrow", tag="tsr")
                nc.sync.dma_start(out=srow[0:1, :], in_=oT0[DH:DH + 1, qo:qo + 512])
                nc.scalar.dma_start(out=srow[1:2, :], in_=oT1[DH:DH + 1, qo:qo + 512])
                rr = []
                for h in range(HL):
                    rs = norm.tile([128, 4], F32, name=f"trs{h}", tag="rs")
                    (nc.sync if h == 0 else nc.scalar).dma_start(
                        out=rs[:], in_=srow[h, :].rearrange("(f p) -> p f", p=128))
                    nc.vector.reciprocal(rs[:], rs[:])
                    rr.append(rs)
                for i, sc in enumerate(range(4 * qc, 4 * qc + 4)):
                    pos = []
                    for h, oT in enumerate((oT0, oT1)):
                        po = auxps.tile([128, 512], F32, name="po", tag="pj")
                        nc.tensor.matmul(
                            po[:, 0:C],
                            lhsT=oT[0:DH, sc * 128:(sc + 1) * 128].bitcast(F32R),
                            rhs=(wo0 if h == 0 else wo1)[:],
                            start=True, stop=True)
                        pos.append(po)
                    ot0 = otsb.tile([128, C], F32, name="ot0", tag="ot")
                    # head 0's scale on the (idle) ACT engine, head 1's
                    # scale+add on DVE — the two run in parallel
                    nc.scalar.activation(ot0[:], pos[0][:, 0:C],
                                         mybir.ActivationFunctionType.Copy,
                                         scale=rr[0][:, i:i + 1])
                    ot = otsb.tile([128, C], F32, name="ot", tag="ot")
                    nc.vector.scalar_tensor_tensor(
                        ot[:], pos[1][:, 0:C], rr[1][:, i:i + 1], ot0[:],
                        mybir.AluOpType.mult, mybir.AluOpType.add)
                    nc.sync.dma_start(out=outp[sc * 128:(sc + 1) * 128, :], in_=ot[:])

            def emit_oproj(sc):
                po = auxps.tile([128, 512], F32, name="po", tag="pj")
                nc.tensor.matmul(po[:, 0:C], lhsT=oTn[:, sc * 128:(sc + 1) * 128],
                                 rhs=wo[:], start=True, stop=True)
                ot = otsb.tile([128, C], F32, name="ot", tag="ot")
                nc.vector.tensor_copy(ot[:], po[:, 0:C])
                nc.sync.dma_start(out=outp[sc * 128:(sc + 1) * 128, :], in_=ot[:])

            # deferred C-tail work, interleaved into the NEXT q-chunk's kv loop
            pending = []

            def pop_pending():
                if pending:
                    pending.pop(0)()

            first = True
            v_emitted = 0
            st_carry = None
            qcs = [q for _ in range(reps) for q in range(NQ)]
            for qi, qc in enumerate(qcs):
                is_last = qi == len(qcs) - 1
                qo = qc * 512
                pva = [pvps.tile([DH + 1, 512], F32, name=f"pvacc{h}",
                                 tag=f"pv{h}") for h in range(HL)]

                def emit_scores(jc, qo=None):
                    # one [128, 1024] tile = h0's 512 q-cols | h1's 512 q-cols
                    qo = qc * 512 if qo is None else qo
                    st = scps.tile([128, 1024], F32, name="st", tag="st")
                    for h in range(HL):
                        hp = h * DH
                        nc.tensor.matmul(
                            st[:, h * 512:(h + 1) * 512],
                            lhsT=kT[hp:hp + DH, jc * 128:(jc + 1) * 128],
                            rhs=qT[hp:hp + DH, qo:qo + 512],
                            start=True,
                            stop=True,
                        )
                    return st

                # software pipeline: scores(jc+1) are emitted BEFORE pv(jc)
                # so the PE never sits behind the exp in program order
                sts = {0: st_carry if st_carry is not None else emit_scores(0)}
                st_carry = None
                for jc in range(NJ):
                    if jc + 1 < NJ:
                        if first:
                            # kT chunk for the next scores, one chunk of lead
                            while k_emitted <= min((jc + 2) // 4 + 1, NQ - 1):
                                proj_qk(kT, wk_sb, k_emitted)
                                k_emitted += 1
                        sts[jc + 1] = emit_scores(jc + 1)
                    elif not is_last:
                        # next q-chunk's first scores, hoisted ahead of the
                        # final pv pair so ACT rolls straight across chunks
                        st_carry = emit_scores(0, qo=qcs[qi + 1] * 512)
                    st = sts.pop(jc)
                    # exp(score/8) for both heads in ONE ACT instruction;
                    # no max pass (scores are O(1))
                    pt = ptsb.tile([128, 1024], F32R, name="pt", tag="pt")
                    nc.scalar.activation(pt[:], st[:], EXP, scale=0.125)
                    if first:
                        # V projection (512-wide chunks) ahead of its use by pv
                        while v_emitted * 4 < min(jc + 4, NJ):
                            proj_v(v_emitted)
                            v_emitted += 1
                    for h, vh in enumerate((v0, v1)):
                        nc.tensor.matmul(
                            pva[h][:],
                            lhsT=vh[:, jc, :],
                            rhs=pt[:, h * 512:(h + 1) * 512],
                            start=(jc == 0),
                            stop=(jc == NJ - 1),
                        )
                    if jc == 10 and deferred_a:
                        deferred_a.pop(0)()
                    if not first and jc in (2, 8, 14, 20, 26):
                        pop_pending()
                if is_last:
                    # f32r-rounded copies: the tail consumes oT via fp32r
                    # matmuls (the BIR verifier requires rounded producers)
                    for h, oT in enumerate((oT0, oT1)):
                        nc.vector.tensor_copy(
                            oT[DH:DH + 1, qo:qo + 512].bitcast(F32R),
                            pva[h][DH:DH + 1, :].bitcast(F32R))
                    for h, oT in enumerate((oT0, oT1)):
                        nc.vector.tensor_copy(
                            oT[0:DH, qo:qo + 512].bitcast(F32R),
                            pva[h][0:DH, :].bitcast(F32R))
                else:
                    for h, oT in enumerate((oT0, oT1)):
                        nc.vector.tensor_copy(oT[:, qo:qo + 512].bitcast(F32R),
                                              pva[h][:].bitcast(F32R))

                first = False

                if is_last:
                    while pending:
                        pop_pending()
                    emit_tail(qc)
                else:
                    pending.append(lambda q=qc: emit_norm(q))
                    for sc in range(4 * qc, 4 * qc + 4):
                        pending.append(lambda sc=sc: emit_oproj(sc))
        stack.close()


def build_nc(reps=1, full_reps=1):
    """full_reps > 1 repeats the ENTIRE kernel (loads + projections +
    attention + output) back-to-back inside one NEFF — used by test.py to
    measure per-execution HW time with host dispatch amortized away."""
    nc = bacc.Bacc("TRN2", target_bir_lowering=False, debug=False)
    hidT = nc.dram_tensor("hidT", [C, S], F32R, kind="ExternalInput").ap()
    wqT = nc.dram_tensor("wqT", [C, DL], F32R, kind="ExternalInput").ap()
    wkT = nc.dram_tensor("wkT", [C, DL], F32R, kind="ExternalInput").ap()
    wvT = nc.dram_tensor("wvT", [C, DL], F32R, kind="ExternalInput").ap()
    woT = nc.dram_tensor("woT", [DL, C], F32R, kind="ExternalInput").ap()
    outp = nc.dram_tensor("outp", [S, C], F32, kind="ExternalOutput").ap()
    with tile.TileContext(nc) as tc:
        for _ in range(full_reps):
            _emit(tc, nc, hidT, wqT, wkT, wvT, woT, outp, reps=reps)
    nc.compile()
    return nc


def make_in_maps(hidden_states, Wq, Wk, Wv, Wo):
    """Shard the full inputs into 8 per-core input maps."""
    hs = np.asarray(hidden_states, dtype=np.float32)
    hidT_b = [np.ascontiguousarray(hs[b].T) for b in range(B)]
    in_maps = []
    for core in range(N_CORES):
        b = core // 4
        p = core % 4
        lo, hi = 2 * p * DH, (2 * p + 2) * DH
        in_maps.append({
            "hidT": hidT_b[b],
            "wqT": np.ascontiguousarray(np.asarray(Wq, np.float32)[lo:hi, :].T),
            "wkT": np.ascontiguousarray(np.asarray(Wk, np.float32)[lo:hi, :].T),
            "wvT": np.ascontiguousarray(np.asarray(Wv, np.float32)[lo:hi, :].T),
            "woT": np.ascontiguousarray(np.asarray(Wo, np.float32)[:, lo:hi].T),
        })
    return in_maps


def gather_output(results, bo):
    """Sum the 4 per-core partial projections per batch, add bias."""
    bo = np.asarray(bo, np.float32)
    out = np.empty((B, S, C), np.float32)
    for b in range(B):
        acc = results[4 * b]["outp"].astype(np.float32).copy()
        for p in range(1, 4):
            acc += results[4 * b + p]["outp"]
        out[b] = acc + bo
    return out


_NC_CACHE = None


def _get_nc():
    global _NC_CACHE
    if _NC_CACHE is None:
        _NC_CACHE = build_nc()
    return _NC_CACHE


def kernel(hidden_states, Wq, Wk, Wv, Wo, bo, _trace=False, _res_out=None):
    nc = _get_nc()
    in_maps = make_in_maps(hidden_states, Wq, Wk, Wv, Wo)
    res = run_bass_kernel_spmd(nc, in_maps, list(range(N_CORES)), trace=_trace)
    if _res_out is not None:
        _res_out.append(res)
    return gather_output(res.results, bo)
